# revision 47
# baseline (speedup 1.0000x reference)
"""ComplexCrossAttention Trainium2 kernel: 8 cores = DP(batch=2) x TP(head-groups=4).

Each core (b = core//4, g = core%4) handles batch b and heads 4g..4g+3.
All matmuls run in bf16 with fp32 PSUM accumulation.

Layout trick: complex arithmetic is folded into the matmul contraction by
packing weights host-side. Per head h the on-chip Q/K layout is
[Qr_h(64 d-rows); Qi_h(64 d-rows)] so that

    scores_h^T = KX_h(.T) @ QX_h = Kr.Qr + Ki.Qi        (one K=128 matmul)

Scores live transposed ([k, q]); the softmax mask is folded into the
denominator matmul's stationary operand (mask value instead of 1.0) and into
a per-k-row scaling of V, so exp needs no bias and runs on 2-bank
[128,1024] PSUM tiles. V is packed as [Vr_h | Vi_h] columns so attn.V is one
M=128 matmul per k-tile; Wo rows are re-ordered to match, and the host adds
the per-core partial Wo outputs (the hint's all-reduce, done host-side).

v2 scheduling: phases K -> V -> Q -> attention with the output projection
interleaved one q-tile behind, to keep the PE continuously busy (it ramps to
full clock only after ~3us without gaps). DMA is spread over three queues
(sync: wk1+x+y, gpsimd: ctx+wv+masks, scalar: wk2+wq+wo) so the x stream
never starves the Q projection.
"""

import numpy as np
import ml_dtypes

import concourse.bacc as bacc
import concourse.mybir as mybir
import concourse.tile as tile
from concourse.bass_utils import run_bass_kernel_spmd

BF16 = ml_dtypes.bfloat16
F32 = mybir.dt.float32
F16 = mybir.dt.float16
BF = mybir.dt.bfloat16

B, S, Lc = 2, 2048, 1024
F, Dc, H = 1024, 768, 16
HD = 64
NCORES = 8
TPG = 4            # head-groups (TP degree per batch)
FS = F // TPG      # 256 features per core
HL = 4             # heads per core
NQ, QTS = 4, 512   # q tiles
NKT = 8            # k tiles of 128 (Lc)
NFIN = 8           # f_in chunks of 128 (Q proj contraction)
NDC = 6            # Dc chunks of 128 (K/V proj contraction)
WW = 2 * HD * HL   # 512 merged (r,i) weight columns per core
SCALE = 1.0 / 8.0  # 1/sqrt(HD)

_CACHE = {}


def _build_nc():
    nc = bacc.Bacc()
    dt = mybir.dt

    # pre-tiled on host: [c, qpair, 128, 2048] with row =
    # [xTr q0 | xTi q0 | xTr q1 | xTi q1]; contiguous => 4KB DMA descriptors
    xT = nc.dram_tensor("xT", [NFIN, NQ // 2, 128, 4 * QTS], dt.bfloat16, kind="ExternalInput")
    cTr = nc.dram_tensor("cTr", [Dc, Lc], dt.bfloat16, kind="ExternalInput")
    cTi = nc.dram_tensor("cTi", [Dc, Lc], dt.bfloat16, kind="ExternalInput")
    w_d = {}
    for n, nch, wid in (
        ("wqr", NFIN, FS), ("wqi", NFIN, FS), ("wqs", NFIN, FS),
        ("wk1", NDC, WW), ("wk2", NDC, WW),
        ("wv1", NDC, WW), ("wv2", NDC, WW),
        ("wo1", HL, F), ("wo2", HL, F),
    ):
        # host-packed [128, nch*wid]: one contiguous DMA per weight tensor
        w_d[n] = nc.dram_tensor(n, [128, nch * wid], dt.bfloat16, kind="ExternalInput")
    # mask per k-row: maskc [128, NKT] fp32 for V row scaling; maskb
    # [128, NKT*128] bf16 (each column block = mask vector) for the
    # denominator matmul's stationary operand.
    maskc_d = nc.dram_tensor("maskc", [128, NKT], dt.float32, kind="ExternalInput")
    maskb_d = nc.dram_tensor("maskb", [128, NKT * 128], dt.bfloat16, kind="ExternalInput")
    yr_d = nc.dram_tensor("yr", [S, F], dt.float16, kind="ExternalOutput")
    yi_d = nc.dram_tensor("yi", [S, F], dt.float16, kind="ExternalOutput")

    EXP = mybir.ActivationFunctionType.Exp

    with tile.TileContext(nc) as tc:
        with (
            tc.tile_pool(name="res", bufs=1) as res,       # kernel-lifetime tiles
            tc.tile_pool(name="xs", bufs=10) as xs,        # streamed xT slices
            tc.tile_pool(name="tw", bufs=15) as tw,        # xr+xi staging (Gauss Q)
            tc.tile_pool(name="ep", bufs=5) as ep,         # exp(scores) tiles
            tc.tile_pool(name="rc", bufs=2) as rc,         # reciprocal staging
            tc.tile_pool(name="ys", bufs=2) as ys,         # y staging
            tc.tile_pool(name="ps", bufs=2, space="PSUM") as ps,    # 2-bank tiles
            tc.tile_pool(name="acc", bufs=4, space="PSUM") as acc,  # 1-bank tiles
        ):
            def rtile(shape, dtype, tag):
                return res.tile(shape, dtype, tag=tag, name=tag)

            # ---- DMA issue (three queues) --------------------------------
            # SWDGE (gpsimd) completion lags ~5us at cold start, so the
            # K-phase inputs ride the two HWDGE queues (sync, scalar), in
            # need-order so the early HBM burst isn't stolen by the x
            # stream. gpsimd only carries the V weights and masks.
            # sync: wk1 chunks + wk2, then (much later) y outputs.
            wk1_c = []
            for c in range(NDC):
                t = rtile([128, WW], BF, f"wk1{c}")
                nc.sync.dma_start(t[:], w_d["wk1"][:, c * WW : (c + 1) * WW])
                wk1_c.append(t)
            wk2 = rtile([128, NDC * WW], BF, "wk2")
            nc.sync.dma_start(wk2[:], w_d["wk2"][:])
            wv_c = {}
            for n in ("wv1", "wv2"):
                for c in range(NDC):
                    t = rtile([128, WW], BF, f"{n}{c}")
                    nc.sync.dma_start(t[:], w_d[n][:, c * WW : (c + 1) * WW])
                    wv_c[n, c] = t

            # scalar queue: ctx chunks, Q weights, x stream, O weights.
            cT_sb = {}
            for name, dram in (("cTr", cTr), ("cTi", cTi)):
                tiles = []
                for c in range(NDC):
                    t = rtile([128, Lc], BF, f"{name}{c}")
                    nc.scalar.dma_start(t[:], dram[c * 128 : (c + 1) * 128, :])
                    tiles.append(t)
                cT_sb[name] = tiles
            w_sb = {}
            for n in ("wqr", "wqi", "wqs"):
                t = rtile([128, NFIN * FS], BF, n)
                nc.scalar.dma_start(t[:], w_d[n][:])
                w_sb[n] = t
            xt_t = {}
            for qp in range(NQ // 2):
                for c in range(NFIN):
                    t = xs.tile([128, 4 * QTS], BF, tag="xt", name="xt")
                    nc.scalar.dma_start(t[:], xT[c, qp])
                    xt_t[qp, c] = t
            for n in ("wo1", "wo2"):
                t = rtile([128, HL * F], BF, n)
                nc.scalar.dma_start(t[:], w_d[n][:])
                w_sb[n] = t

            # gpsimd: masks only (tiny; SWDGE cold-start lag is harmless).
            maskc = rtile([128, NKT], F32, "maskc")
            nc.gpsimd.dma_start(maskc[:], maskc_d[:])
            maskb = rtile([128, NKT * 128], BF, "maskb")
            nc.gpsimd.dma_start(maskb[:], maskb_d[:])

            # merged per-head tiles: rows = [comp_r d(64); comp_i d(64)]
            QX = {h: rtile([128, S], BF, f"qx{h}") for h in range(HL)}
            KX = {h: rtile([128, Lc], BF, f"kx{h}") for h in range(HL)}
            Vsb = {kt: rtile([128, WW], BF, f"v{kt}") for kt in range(NKT)}
            # OT is a 2-q-tile ring: the output projection lags attention by
            # exactly one q-tile, so only the current and previous q-tile's
            # attention outputs are ever live.
            OT = {h: rtile([128, 2 * QTS], BF, f"ot{h}") for h in range(HL)}

            # ---- K projection --------------------------------------------
            # All 8 psums (both kq halves) accumulate together, one full
            # round per arriving ctx chunk (8 matmuls), so the PE keeps pace
            # with the cold-start DMA arrival rate. kq0 lives in the 2-bank
            # ps pool (head-pair tiles), kq1 in four 1-bank acc tiles.
            pk0 = {hp: ps.tile([128, 1024], F32, tag="ps", name="ps")
                   for hp in range(2)}
            pk1 = {h: acc.tile([128, 512], F32, tag="acc", name="acc")
                   for h in range(HL)}
            for first, ct in ((True, "cTr"), (False, "cTi")):
                for c in range(NDC):
                    wsl = wk1_c[c] if first else wk2
                    base = 0 if first else c * WW
                    for h in range(HL):
                        w_ap = wsl[:, base + h * 128 : base + (h + 1) * 128]
                        nc.tensor.matmul(
                            pk0[h // 2][:, (h % 2) * 512 : (h % 2 + 1) * 512],
                            w_ap, cT_sb[ct][c][:, 0:512],
                            start=(first and c == 0),
                            stop=(not first and c == NDC - 1),
                        )
                        nc.tensor.matmul(
                            pk1[h][:], w_ap, cT_sb[ct][c][:, 512:1024],
                            start=(first and c == 0),
                            stop=(not first and c == NDC - 1),
                        )
            for h in range(HL):
                nc.vector.tensor_copy(
                    KX[h][:, 0:512], pk0[h // 2][:, (h % 2) * 512 : (h % 2 + 1) * 512]
                )
                nc.vector.tensor_copy(KX[h][:, 512:1024], pk1[h][:])

            # ---- V projection (natural [k, d]; columns [Vr_h | Vi_h] x4) --
            # Same full-round structure: kt0-3 in ps-pool pairs, kt4-7 in
            # acc tiles; mask folded in via per-k-row scaling.
            pv0 = {i: ps.tile([128, 1024], F32, tag="ps", name="ps")
                   for i in range(2)}
            pv1 = {kt: acc.tile([128, 512], F32, tag="acc", name="acc")
                   for kt in range(4, NKT)}

            def pv_ap(kt):
                return (pv0[kt // 2][:, (kt % 2) * 512 : (kt % 2 + 1) * 512]
                        if kt < 4 else pv1[kt][:])

            # first half c-round-robin (paced by chunk arrival at cold start)
            for c in range(NDC):
                for kt in range(NKT):
                    nc.tensor.matmul(
                        pv_ap(kt), cT_sb["cTr"][c][:, kt * 128 : (kt + 1) * 128],
                        wv_c["wv1", c][:], start=(c == 0), stop=False,
                    )
            # second half kt-major so psums close staggered; kt4-7 (acc pool)
            # first since the Q projection reuses those slots next.
            for kt in list(range(4, NKT)) + list(range(4)):
                for c in range(NDC):
                    nc.tensor.matmul(
                        pv_ap(kt), cT_sb["cTi"][c][:, kt * 128 : (kt + 1) * 128],
                        wv_c["wv2", c][:], start=False, stop=(c == NDC - 1),
                    )
                nc.vector.tensor_scalar_mul(
                    Vsb[kt][:], pv_ap(kt), maskc[:, kt : kt + 1]
                )

            # ---- Q projection (Gauss 3-multiplication) -------------------
            # Qr = M1 - M2, Qi = M3 - M1 - M2 with M1 = xr@Wqr, M2 = xi@Wqi,
            # M3 = (xr+xi)@(Wqr+Wqi): 48 matmuls per q-tile instead of 64;
            # the combines run on the otherwise-idle vector engine. Psum
            # roles: M1 head-pairs on the 2-bank ps pool (ring slack), M2/M3
            # in acc, so wave-to-wave reuse stalls stay under ~1us.
            for qp in range(NQ // 2):
                for qh in range(2):
                    q = 2 * qp + qh
                    qs = slice(q * QTS, (q + 1) * QTS)
                    xr = {c: xt_t[qp, c][:, 2 * qh * QTS : (2 * qh + 1) * QTS]
                          for c in range(NFIN)}
                    xi = {c: xt_t[qp, c][:, (2 * qh + 1) * QTS : (2 * qh + 2) * QTS]
                          for c in range(NFIN)}
                    # xr+xi staged on the otherwise-idle gpsimd engine (the
                    # vector queue must not block behind late x DMAs, and the
                    # 16-deep tw pool lets gpsimd run ~1.5 waves ahead of the
                    # PE, which it needs since its adds are slightly slower
                    # than the wave rate).
                    xm = {}
                    for c in range(NFIN):
                        t = tw.tile([128, QTS], BF, tag="xm", name="xm")
                        nc.gpsimd.tensor_add(t[:], xr[c], xi[c])
                        xm[c] = t[:]
                    # M1/M2 head-pairs on the ps pool (slots recycle fast:
                    # M1's only reader is the SBUF copy), M3 in acc where two
                    # waves fit, so wave-to-wave psum reuse never stalls.
                    m1 = ps.tile([128, 1024], F32, tag="ps", name="ps")
                    m2t = ps.tile([128, 1024], F32, tag="ps", name="ps")
                    m2 = {hp: m2t[:, hp * 512 : (hp + 1) * 512] for hp in range(2)}
                    m3 = {hp: acc.tile([128, QTS], F32, tag="acc", name="acc")
                          for hp in range(2)}
                    for wn, dst, src in (
                        ("wqr", lambda hp: m1[:, hp * 512 : (hp + 1) * 512], xr),
                        ("wqi", lambda hp: m2[hp], xi),
                        ("wqs", lambda hp: m3[hp][:], xm),
                    ):
                        for c in range(NFIN):
                            for hp in range(2):
                                nc.tensor.matmul(
                                    dst(hp),
                                    w_sb[wn][:, c * FS + hp * 128 : c * FS + (hp + 1) * 128],
                                    src[c],
                                    start=(c == 0), stop=(c == NFIN - 1),
                                )
                    # PSUM has one DVE read port, so M1 is staged to SBUF
                    # once and every sub then reads one PSUM + one SBUF
                    # operand. DVE lanes are partition-locked, so the two
                    # crossed halves go through an aligned scratch tile +
                    # partition-shifting SBUF->SBUF DMA (idle sync queue).
                    for hp in range(2):
                        h0, h1 = 2 * hp, 2 * hp + 1
                        m1sb = rc.tile([128, QTS], F32, tag="rc", name="rc")
                        nc.vector.tensor_copy(
                            m1sb[:], m1[:, hp * 512 : (hp + 1) * 512])
                        im = rc.tile([128, QTS], F32, tag="rc", name="rc")
                        nc.vector.tensor_sub(im[:], m3[hp][:], m1sb[:])
                        sc = tw.tile([128, QTS], BF, tag="sc", name="sc")
                        m2lo = m2t[0:64, hp * 512 : (hp + 1) * 512]
                        m2hi = m2t[64:128, hp * 512 : (hp + 1) * 512]
                        # aligned: Qr_h0 (rows 0:64), Qi_h1 (rows 64:128)
                        nc.vector.tensor_sub(
                            QX[h0][0:64, qs], m1sb[0:64, :], m2lo)
                        nc.vector.tensor_sub(
                            QX[h1][64:128, qs], im[64:128, :], m2hi)
                        # crossed: Qi_h0 (psum rows 0:64 -> QX rows 64:128),
                        # Qr_h1 (psum rows 64:128 -> QX rows 0:64)
                        nc.vector.tensor_sub(
                            sc[0:64, :], im[0:64, :], m2lo)
                        nc.vector.tensor_sub(
                            sc[64:128, :], m1sb[64:128, :], m2hi)
                        nc.sync.dma_start(QX[h0][64:128, qs], sc[0:64, :])
                        nc.sync.dma_start(QX[h1][0:64, qs], sc[64:128, :])

            # ---- attention + interleaved output projection ---------------
            # One continuous stream over (q, h, double-k-tile) with dn/av
            # lagging scores by LAG double-k-tiles ACROSS step boundaries, so
            # the PE never waits on the scalar engine's exp. The output
            # projection for q-tile q-1 is interleaved one step behind and
            # accumulates on 2-bank ps-pool tiles (never contends with the
            # dn/av pool).
            NDK = NKT // 2  # double k-tiles per (h, q)
            LAG = 2

            def og_emit(qig):
                """Output projection for one 128-row q block (all 4 heads)."""
                qsl = slice(qig * 128, (qig + 1) * 128)
                ot_col = ((qig // 4) % 2) * 512 + (qig % 4) * 128
                osl = slice(ot_col, ot_col + 128)
                for wname, dram in (("wo1", yr_d), ("wo2", yi_d)):
                    op = ps.tile([128, 1024], F32, tag="ps", name="ps")
                    st = ys.tile([128, F], F16, tag="y", name="y")
                    for fo in range(2):
                        for h in range(HL):
                            nc.tensor.matmul(
                                op[:, fo * 512 : (fo + 1) * 512],
                                OT[h][:, osl],
                                w_sb[wname][:, h * F + fo * 512 : h * F + (fo + 1) * 512],
                                start=(h == 0),
                                stop=(h == HL - 1),
                            )
                        # cast each half as soon as its accumulation closes
                        nc.vector.tensor_copy(
                            st[:, fo * 512 : (fo + 1) * 512],
                            op[:, fo * 512 : (fo + 1) * 512],
                        )
                    nc.sync.dma_start(dram[qsl, :], st[:])

            state = {}

            def scores_and_exp(si, dkt):
                q, h = si // HL, si % HL
                sp = ps.tile([128, 1024], F32, tag="ps", name="ps")
                for half in range(2):
                    kt = 2 * dkt + half
                    nc.tensor.matmul(
                        sp[:, half * 512 : (half + 1) * 512],
                        KX[h][:, kt * 128 : (kt + 1) * 128],
                        QX[h][:, q * QTS : (q + 1) * QTS],
                        start=True, stop=True,
                    )
                e = ep.tile([128, 1024], BF, tag="e", name="e")
                nc.scalar.activation(e[:], sp[:], EXP, bias=0.0, scale=SCALE)
                state.setdefault(si, {})[dkt] = e

            def dn_av(si, dkt):
                q, h = si // HL, si % HL
                st = state[si]
                if dkt == 0:
                    st["dn"] = acc.tile([128, QTS], F32, tag="acc", name="acc")
                    st["av"] = acc.tile([128, QTS], F32, tag="acc", name="acc")
                e = st.pop(dkt)
                vsl = slice(h * 128, (h + 1) * 128)
                for half in range(2):
                    kt = 2 * dkt + half
                    first = dkt == 0 and half == 0
                    last = dkt == NDK - 1 and half == 1
                    esl = e[:, half * 512 : (half + 1) * 512]
                    nc.tensor.matmul(
                        st["dn"][:], maskb[:, kt * 128 : (kt + 1) * 128], esl,
                        start=first, stop=last,
                    )
                    nc.tensor.matmul(
                        st["av"][:], Vsb[kt][:, vsl], esl,
                        start=first, stop=last,
                    )
                if dkt == NDK - 1:
                    rec = rc.tile([128, QTS], F32, tag="rc", name="rc")
                    nc.vector.reciprocal_approx_fast(rec[:], st["dn"][:])
                    nc.vector.tensor_mul(
                        OT[h][:, (q % 2) * QTS : (q % 2 + 1) * QTS],
                        st["av"][:], rec[:]
                    )
                    del state[si]

            tasks = []
            for si in range(NQ * HL):
                for dkt in range(NDK):
                    scores_and_exp(si, dkt)
                    tasks.append((si, dkt))
                    if len(tasks) > LAG:
                        dn_av(*tasks.pop(0))
                if si >= HL:
                    og_emit(si - HL)
            while tasks:
                dn_av(*tasks.pop(0))
            for si in range(NQ * HL - HL, NQ * HL):
                og_emit(si)

    nc.compile()
    return nc


def _prep_in_maps(inputs):
    f32 = np.float32

    def bf(a):
        return np.ascontiguousarray(a).astype(BF16)

    x_r, x_i = np.asarray(inputs["x_r"], f32), np.asarray(inputs["x_i"], f32)
    ctx_r, ctx_i = np.asarray(inputs["ctx_r"], f32), np.asarray(inputs["ctx_i"], f32)
    mask = np.asarray(inputs["mask"], f32)
    W = {k: np.asarray(inputs[k], f32) for k in
         ("Wqr", "Wqi", "Wkr", "Wki", "Wvr", "Wvi", "Wor", "Woi")}

    per_batch = {}
    for b in range(B):
        def xtile(a):
            # [S, F] -> [F, S] -> [NFIN, NQ, 128, 512]
            return a.T.reshape(NFIN, 128, NQ, QTS).transpose(0, 2, 1, 3)

        tr, ti = xtile(x_r[b]), xtile(x_i[b])
        # [NFIN, NQ, 128, 2*QTS] with (r|i) per q, then fold q-pairs into rows
        xri = np.concatenate([tr, ti], axis=-1)
        xri = (
            xri.reshape(NFIN, NQ // 2, 2, 128, 2 * QTS)
            .transpose(0, 1, 3, 2, 4)
            .reshape(NFIN, NQ // 2, 128, 4 * QTS)
        )

        mcol = mask[b].reshape(NKT, 128).T  # [128, NKT]
        per_batch[b] = {
            "xT": bf(xri),
            "cTr": bf(ctx_r[b].T),
            "cTi": bf(ctx_i[b].T),
            "maskc": np.ascontiguousarray(mcol.astype(f32)),
            "maskb": bf(np.repeat(mcol, 128, axis=1)),
        }

    def merge_cols(Wr, Wi, g):
        """[Din, F] pair -> per-head merged column blocks.

        Returns (w1, w2) of shape [Din, HL*128]: per head h the 128 columns
        are [comp1_h(64) | comp2_h(64)] with w1 = [Wr_h | Wi_h] and
        w2 = [-Wi_h | Wr_h], so psum = w1^T xr + w2^T xi yields rows
        [real_h; imag_h]."""
        din = Wr.shape[0]
        w1 = np.empty((din, HL * 128), f32)
        w2 = np.empty((din, HL * 128), f32)
        for h in range(HL):
            cs = slice(g * FS + h * HD, g * FS + (h + 1) * HD)
            w1[:, h * 128 : h * 128 + 64] = Wr[:, cs]
            w1[:, h * 128 + 64 : (h + 1) * 128] = Wi[:, cs]
            w2[:, h * 128 : h * 128 + 64] = -Wi[:, cs]
            w2[:, h * 128 + 64 : (h + 1) * 128] = Wr[:, cs]
        return w1, w2

    def pack(w, nch, wid):
        # [nch*128, wid] -> packed [128, nch*wid]
        return bf(w.reshape(nch, 128, wid).transpose(1, 0, 2).reshape(128, -1))

    in_maps = []
    for core in range(NCORES):
        b, g = core // TPG, core % TPG
        m = dict(per_batch[b])
        # Gauss Q weights: plain per-core column slices of Wqr/Wqi/(Wqr+Wqi)
        gs = slice(g * FS, (g + 1) * FS)
        m["wqr"] = pack(W["Wqr"][:, gs], NFIN, FS)
        m["wqi"] = pack(W["Wqi"][:, gs], NFIN, FS)
        m["wqs"] = pack(W["Wqr"][:, gs] + W["Wqi"][:, gs], NFIN, FS)
        for pre, wr, wi, nch in (
            ("wk", "Wkr", "Wki", NDC),
            ("wv", "Wvr", "Wvi", NDC),
        ):
            w1, w2 = merge_cols(W[wr], W[wi], g)
            m[pre + "1"] = pack(w1, nch, WW)
            m[pre + "2"] = pack(w2, nch, WW)
        # Wo: rows re-ordered to the merged [out_r_h(64); out_i_h(64)] layout.
        wo1 = np.empty((HL, 128, F), f32)
        wo2 = np.empty((HL, 128, F), f32)
        for h in range(HL):
            rs = slice(g * FS + h * HD, g * FS + (h + 1) * HD)
            wo1[h, :64] = W["Wor"][rs]
            wo1[h, 64:] = -W["Woi"][rs]
            wo2[h, :64] = W["Woi"][rs]
            wo2[h, 64:] = W["Wor"][rs]
        m["wo1"] = bf(wo1.transpose(1, 0, 2).reshape(128, -1))
        m["wo2"] = bf(wo2.transpose(1, 0, 2).reshape(128, -1))
        in_maps.append(m)
    return in_maps


def kernel(**inputs):
    if "nc" not in _CACHE:
        _CACHE["nc"] = _build_nc()
    nc = _CACHE["nc"]
    in_maps = _prep_in_maps(inputs)
    res = run_bass_kernel_spmd(nc, in_maps, core_ids=list(range(NCORES)))
    y = np.zeros((B, S, F), np.complex64)
    for core in range(NCORES):
        b = core // TPG
        y[b] += res.results[core]["yr"].astype(np.float32)
        y[b] += 1j * res.results[core]["yi"].astype(np.float32)
    return y


# revision 48
# speedup vs baseline: 1.0174x; 1.0174x over previous
"""ComplexCrossAttention Trainium2 kernel: 8 cores = DP(batch=2) x TP(head-groups=4).

Each core (b = core//4, g = core%4) handles batch b and heads 4g..4g+3.
All matmuls run in bf16 with fp32 PSUM accumulation.

Layout trick: complex arithmetic is folded into the matmul contraction by
packing weights host-side. Per head h the on-chip Q/K layout is
[Qr_h(64 d-rows); Qi_h(64 d-rows)] so that

    scores_h^T = KX_h(.T) @ QX_h = Kr.Qr + Ki.Qi        (one K=128 matmul)

Scores live transposed ([k, q]); the softmax mask is folded into the
denominator matmul's stationary operand (mask value instead of 1.0) and into
a per-k-row scaling of V, so exp needs no bias and runs on 2-bank
[128,1024] PSUM tiles. V is packed as [Vr_h | Vi_h] columns so attn.V is one
M=128 matmul per k-tile; Wo rows are re-ordered to match, and the host adds
the per-core partial Wo outputs (the hint's all-reduce, done host-side).

v2 scheduling: phases K -> V -> Q -> attention with the output projection
interleaved one q-tile behind, to keep the PE continuously busy (it ramps to
full clock only after ~3us without gaps). DMA is spread over three queues
(sync: wk1+x+y, gpsimd: ctx+wv+masks, scalar: wk2+wq+wo) so the x stream
never starves the Q projection.
"""

import numpy as np
import ml_dtypes

import concourse.bacc as bacc
import concourse.mybir as mybir
import concourse.tile as tile
from concourse.bass_utils import run_bass_kernel_spmd

BF16 = ml_dtypes.bfloat16
F32 = mybir.dt.float32
F16 = mybir.dt.float16
BF = mybir.dt.bfloat16

B, S, Lc = 2, 2048, 1024
F, Dc, H = 1024, 768, 16
HD = 64
NCORES = 8
TPG = 4            # head-groups (TP degree per batch)
FS = F // TPG      # 256 features per core
HL = 4             # heads per core
NQ, QTS = 4, 512   # q tiles
NKT = 8            # k tiles of 128 (Lc)
NFIN = 8           # f_in chunks of 128 (Q proj contraction)
NDC = 6            # Dc chunks of 128 (K/V proj contraction)
WW = 2 * HD * HL   # 512 merged (r,i) weight columns per core
SCALE = 1.0 / 8.0  # 1/sqrt(HD)

_CACHE = {}


def _build_nc():
    nc = bacc.Bacc()
    dt = mybir.dt

    # pre-tiled on host: [c, qpair, 128, 2048] with row =
    # [xTr q0 | xTi q0 | xTr q1 | xTi q1]; contiguous => 4KB DMA descriptors
    xT = nc.dram_tensor("xT", [NFIN, NQ // 2, 128, 4 * QTS], dt.bfloat16, kind="ExternalInput")
    cTr = nc.dram_tensor("cTr", [Dc, Lc], dt.bfloat16, kind="ExternalInput")
    cTi = nc.dram_tensor("cTi", [Dc, Lc], dt.bfloat16, kind="ExternalInput")
    w_d = {}
    for n, nch, wid in (
        ("wqr", NFIN, FS), ("wqi", NFIN, FS), ("wqs", NFIN, FS),
        ("wk1", NDC, WW), ("wk2", NDC, WW),
        ("wv1", NDC, WW), ("wv2", NDC, WW),
        ("wo1", HL, F), ("wo2", HL, F),
    ):
        # host-packed [128, nch*wid]: one contiguous DMA per weight tensor
        w_d[n] = nc.dram_tensor(n, [128, nch * wid], dt.bfloat16, kind="ExternalInput")
    # mask per k-row: maskc [128, NKT] fp32 for V row scaling; maskb
    # [128, NKT*128] bf16 (each column block = mask vector) for the
    # denominator matmul's stationary operand.
    maskc_d = nc.dram_tensor("maskc", [128, NKT], dt.float32, kind="ExternalInput")
    maskb_d = nc.dram_tensor("maskb", [128, NKT * 128], dt.bfloat16, kind="ExternalInput")
    yr_d = nc.dram_tensor("yr", [S, F], dt.float16, kind="ExternalOutput")
    yi_d = nc.dram_tensor("yi", [S, F], dt.float16, kind="ExternalOutput")

    EXP = mybir.ActivationFunctionType.Exp

    with tile.TileContext(nc) as tc:
        with (
            tc.tile_pool(name="res", bufs=1) as res,       # kernel-lifetime tiles
            tc.tile_pool(name="xs", bufs=10) as xs,        # streamed xT slices
            tc.tile_pool(name="tw", bufs=11) as tw,        # xr+xi staging (Gauss Q)
            tc.tile_pool(name="ep", bufs=6) as ep,         # exp(scores) tiles
            tc.tile_pool(name="rc", bufs=2) as rc,         # reciprocal staging
            tc.tile_pool(name="ys", bufs=3) as ys,         # y staging
            tc.tile_pool(name="ps", bufs=2, space="PSUM") as ps,    # 2-bank tiles
            tc.tile_pool(name="acc", bufs=4, space="PSUM") as acc,  # 1-bank tiles
        ):
            def rtile(shape, dtype, tag):
                return res.tile(shape, dtype, tag=tag, name=tag)

            # ---- DMA issue (three queues) --------------------------------
            # SWDGE (gpsimd) completion lags ~5us at cold start, so the
            # K-phase inputs ride the two HWDGE queues (sync, scalar), in
            # need-order so the early HBM burst isn't stolen by the x
            # stream. gpsimd only carries the V weights and masks.
            # sync: wk1 chunks + wk2, then (much later) y outputs.
            wk1_c = []
            for c in range(NDC):
                t = rtile([128, WW], BF, f"wk1{c}")
                nc.sync.dma_start(t[:], w_d["wk1"][:, c * WW : (c + 1) * WW])
                wk1_c.append(t)
            wk2 = rtile([128, NDC * WW], BF, "wk2")
            nc.sync.dma_start(wk2[:], w_d["wk2"][:])
            wv_c = {}
            for n in ("wv1", "wv2"):
                for c in range(NDC):
                    t = rtile([128, WW], BF, f"{n}{c}")
                    nc.sync.dma_start(t[:], w_d[n][:, c * WW : (c + 1) * WW])
                    wv_c[n, c] = t

            # scalar queue: ctx chunks, Q weights, x stream, O weights.
            cT_sb = {}
            for name, dram in (("cTr", cTr), ("cTi", cTi)):
                tiles = []
                for c in range(NDC):
                    t = rtile([128, Lc], BF, f"{name}{c}")
                    nc.scalar.dma_start(t[:], dram[c * 128 : (c + 1) * 128, :])
                    tiles.append(t)
                cT_sb[name] = tiles
            w_sb = {}
            for n in ("wqr", "wqi", "wqs"):
                t = rtile([128, NFIN * FS], BF, n)
                nc.scalar.dma_start(t[:], w_d[n][:])
                w_sb[n] = t
            xt_t = {}
            for qp in range(NQ // 2):
                for c in range(NFIN):
                    t = xs.tile([128, 4 * QTS], BF, tag="xt", name="xt")
                    nc.scalar.dma_start(t[:], xT[c, qp])
                    xt_t[qp, c] = t
            for n in ("wo1", "wo2"):
                t = rtile([128, HL * F], BF, n)
                nc.scalar.dma_start(t[:], w_d[n][:])
                w_sb[n] = t

            # gpsimd: masks only (tiny; SWDGE cold-start lag is harmless).
            maskc = rtile([128, NKT], F32, "maskc")
            nc.gpsimd.dma_start(maskc[:], maskc_d[:])
            maskb = rtile([128, NKT * 128], BF, "maskb")
            nc.gpsimd.dma_start(maskb[:], maskb_d[:])

            # merged per-head tiles: rows = [comp_r d(64); comp_i d(64)]
            QX = {h: rtile([128, S], BF, f"qx{h}") for h in range(HL)}
            KX = {h: rtile([128, Lc], BF, f"kx{h}") for h in range(HL)}
            Vsb = {kt: rtile([128, WW], BF, f"v{kt}") for kt in range(NKT)}
            # OT is a 2-q-tile ring: the output projection lags attention by
            # exactly one q-tile, so only the current and previous q-tile's
            # attention outputs are ever live.
            OT = {h: rtile([128, 2 * QTS], BF, f"ot{h}") for h in range(HL)}

            # ---- K projection --------------------------------------------
            # All 8 psums (both kq halves) accumulate together, one full
            # round per arriving ctx chunk (8 matmuls), so the PE keeps pace
            # with the cold-start DMA arrival rate. kq0 lives in the 2-bank
            # ps pool (head-pair tiles), kq1 in four 1-bank acc tiles.
            pk0 = {hp: ps.tile([128, 1024], F32, tag="ps", name="ps")
                   for hp in range(2)}
            pk1 = {h: acc.tile([128, 512], F32, tag="acc", name="acc")
                   for h in range(HL)}
            for first, ct in ((True, "cTr"), (False, "cTi")):
                for c in range(NDC):
                    wsl = wk1_c[c] if first else wk2
                    base = 0 if first else c * WW
                    for h in range(HL):
                        w_ap = wsl[:, base + h * 128 : base + (h + 1) * 128]
                        nc.tensor.matmul(
                            pk0[h // 2][:, (h % 2) * 512 : (h % 2 + 1) * 512],
                            w_ap, cT_sb[ct][c][:, 0:512],
                            start=(first and c == 0),
                            stop=(not first and c == NDC - 1),
                        )
                        nc.tensor.matmul(
                            pk1[h][:], w_ap, cT_sb[ct][c][:, 512:1024],
                            start=(first and c == 0),
                            stop=(not first and c == NDC - 1),
                        )
            for h in range(HL):
                nc.vector.tensor_copy(
                    KX[h][:, 0:512], pk0[h // 2][:, (h % 2) * 512 : (h % 2 + 1) * 512]
                )
                nc.vector.tensor_copy(KX[h][:, 512:1024], pk1[h][:])

            # ---- V projection (natural [k, d]; columns [Vr_h | Vi_h] x4) --
            # Same full-round structure: kt0-3 in ps-pool pairs, kt4-7 in
            # acc tiles; mask folded in via per-k-row scaling.
            pv0 = {i: ps.tile([128, 1024], F32, tag="ps", name="ps")
                   for i in range(2)}
            pv1 = {kt: acc.tile([128, 512], F32, tag="acc", name="acc")
                   for kt in range(4, NKT)}

            def pv_ap(kt):
                return (pv0[kt // 2][:, (kt % 2) * 512 : (kt % 2 + 1) * 512]
                        if kt < 4 else pv1[kt][:])

            # first half c-round-robin (paced by chunk arrival at cold start)
            for c in range(NDC):
                for kt in range(NKT):
                    nc.tensor.matmul(
                        pv_ap(kt), cT_sb["cTr"][c][:, kt * 128 : (kt + 1) * 128],
                        wv_c["wv1", c][:], start=(c == 0), stop=False,
                    )
            # second half kt-major so psums close staggered; kt4-7 (acc pool)
            # first since the Q projection reuses those slots next.
            for kt in list(range(4, NKT)) + list(range(4)):
                for c in range(NDC):
                    nc.tensor.matmul(
                        pv_ap(kt), cT_sb["cTi"][c][:, kt * 128 : (kt + 1) * 128],
                        wv_c["wv2", c][:], start=False, stop=(c == NDC - 1),
                    )
                nc.vector.tensor_scalar_mul(
                    Vsb[kt][:], pv_ap(kt), maskc[:, kt : kt + 1]
                )

            # ---- Q projection (Gauss 3-multiplication) -------------------
            # Qr = M1 - M2, Qi = M3 - M1 - M2 with M1 = xr@Wqr, M2 = xi@Wqi,
            # M3 = (xr+xi)@(Wqr+Wqi): 48 matmuls per q-tile instead of 64;
            # the combines run on the otherwise-idle vector engine. Psum
            # roles: M1 head-pairs on the 2-bank ps pool (ring slack), M2/M3
            # in acc, so wave-to-wave reuse stalls stay under ~1us.
            for qp in range(NQ // 2):
                for qh in range(2):
                    q = 2 * qp + qh
                    qs = slice(q * QTS, (q + 1) * QTS)
                    xr = {c: xt_t[qp, c][:, 2 * qh * QTS : (2 * qh + 1) * QTS]
                          for c in range(NFIN)}
                    xi = {c: xt_t[qp, c][:, (2 * qh + 1) * QTS : (2 * qh + 2) * QTS]
                          for c in range(NFIN)}
                    # xr+xi staged on the otherwise-idle gpsimd engine (the
                    # vector queue must not block behind late x DMAs, and the
                    # 16-deep tw pool lets gpsimd run ~1.5 waves ahead of the
                    # PE, which it needs since its adds are slightly slower
                    # than the wave rate).
                    xm = {}
                    for c in range(NFIN):
                        t = tw.tile([128, QTS], BF, tag="xm", name="xm")
                        nc.gpsimd.tensor_add(t[:], xr[c], xi[c])
                        xm[c] = t[:]
                    # M1/M2 head-pairs on the ps pool (slots recycle fast:
                    # M1's only reader is the SBUF copy), M3 in acc where two
                    # waves fit, so wave-to-wave psum reuse never stalls.
                    m1 = ps.tile([128, 1024], F32, tag="ps", name="ps")
                    m2t = ps.tile([128, 1024], F32, tag="ps", name="ps")
                    m2 = {hp: m2t[:, hp * 512 : (hp + 1) * 512] for hp in range(2)}
                    m3 = {hp: acc.tile([128, QTS], F32, tag="acc", name="acc")
                          for hp in range(2)}
                    for wn, dst, src in (
                        ("wqr", lambda hp: m1[:, hp * 512 : (hp + 1) * 512], xr),
                        ("wqi", lambda hp: m2[hp], xi),
                        ("wqs", lambda hp: m3[hp][:], xm),
                    ):
                        for c in range(NFIN):
                            for hp in range(2):
                                nc.tensor.matmul(
                                    dst(hp),
                                    w_sb[wn][:, c * FS + hp * 128 : c * FS + (hp + 1) * 128],
                                    src[c],
                                    start=(c == 0), stop=(c == NFIN - 1),
                                )
                    # PSUM has one DVE read port, so M1 is staged to SBUF
                    # once and every sub then reads one PSUM + one SBUF
                    # operand. DVE lanes are partition-locked, so the two
                    # crossed halves go through an aligned scratch tile +
                    # partition-shifting SBUF->SBUF DMA (idle sync queue).
                    for hp in range(2):
                        h0, h1 = 2 * hp, 2 * hp + 1
                        m1sb = rc.tile([128, QTS], F32, tag="rc", name="rc")
                        nc.vector.tensor_copy(
                            m1sb[:], m1[:, hp * 512 : (hp + 1) * 512])
                        im = rc.tile([128, QTS], F32, tag="rc", name="rc")
                        nc.vector.tensor_sub(im[:], m3[hp][:], m1sb[:])
                        sc = tw.tile([128, QTS], BF, tag="sc", name="sc")
                        m2lo = m2t[0:64, hp * 512 : (hp + 1) * 512]
                        m2hi = m2t[64:128, hp * 512 : (hp + 1) * 512]
                        # aligned: Qr_h0 (rows 0:64), Qi_h1 (rows 64:128)
                        nc.vector.tensor_sub(
                            QX[h0][0:64, qs], m1sb[0:64, :], m2lo)
                        nc.vector.tensor_sub(
                            QX[h1][64:128, qs], im[64:128, :], m2hi)
                        # crossed: Qi_h0 (psum rows 0:64 -> QX rows 64:128),
                        # Qr_h1 (psum rows 64:128 -> QX rows 0:64)
                        nc.vector.tensor_sub(
                            sc[0:64, :], im[0:64, :], m2lo)
                        nc.vector.tensor_sub(
                            sc[64:128, :], m1sb[64:128, :], m2hi)
                        nc.sync.dma_start(QX[h0][64:128, qs], sc[0:64, :])
                        nc.sync.dma_start(QX[h1][0:64, qs], sc[64:128, :])

            # ---- attention + interleaved output projection ---------------
            # One continuous stream over (q, h, double-k-tile) with dn/av
            # lagging scores by LAG double-k-tiles ACROSS step boundaries, so
            # the PE never waits on the scalar engine's exp. The output
            # projection for q-tile q-1 is interleaved one step behind and
            # accumulates on 2-bank ps-pool tiles (never contends with the
            # dn/av pool).
            NDK = NKT // 2  # double k-tiles per (h, q)
            LAG = 2

            def og_emit(qig):
                """Output projection for one 128-row q block (all 4 heads)."""
                qsl = slice(qig * 128, (qig + 1) * 128)
                ot_col = ((qig // 4) % 2) * 512 + (qig % 4) * 128
                osl = slice(ot_col, ot_col + 128)
                for wname, dram in (("wo1", yr_d), ("wo2", yi_d)):
                    op = ps.tile([128, 1024], F32, tag="ps", name="ps")
                    st = ys.tile([128, F], F16, tag="y", name="y")
                    for fo in range(2):
                        for h in range(HL):
                            nc.tensor.matmul(
                                op[:, fo * 512 : (fo + 1) * 512],
                                OT[h][:, osl],
                                w_sb[wname][:, h * F + fo * 512 : h * F + (fo + 1) * 512],
                                start=(h == 0),
                                stop=(h == HL - 1),
                            )
                        # cast each half as soon as its accumulation closes
                        nc.vector.tensor_copy(
                            st[:, fo * 512 : (fo + 1) * 512],
                            op[:, fo * 512 : (fo + 1) * 512],
                        )
                    nc.sync.dma_start(dram[qsl, :], st[:])

            state = {}

            def scores_and_exp(si, dkt):
                q, h = si // HL, si % HL
                sp = ps.tile([128, 1024], F32, tag="ps", name="ps")
                for half in range(2):
                    kt = 2 * dkt + half
                    nc.tensor.matmul(
                        sp[:, half * 512 : (half + 1) * 512],
                        KX[h][:, kt * 128 : (kt + 1) * 128],
                        QX[h][:, q * QTS : (q + 1) * QTS],
                        start=True, stop=True,
                    )
                e = ep.tile([128, 1024], BF, tag="e", name="e")
                nc.scalar.activation(e[:], sp[:], EXP, bias=0.0, scale=SCALE)
                state.setdefault(si, {})[dkt] = e

            def dn_av(si, dkt):
                q, h = si // HL, si % HL
                st = state[si]
                if dkt == 0:
                    st["dn"] = acc.tile([128, QTS], F32, tag="acc", name="acc")
                    st["av"] = acc.tile([128, QTS], F32, tag="acc", name="acc")
                e = st.pop(dkt)
                vsl = slice(h * 128, (h + 1) * 128)
                for half in range(2):
                    kt = 2 * dkt + half
                    first = dkt == 0 and half == 0
                    last = dkt == NDK - 1 and half == 1
                    esl = e[:, half * 512 : (half + 1) * 512]
                    nc.tensor.matmul(
                        st["dn"][:], maskb[:, kt * 128 : (kt + 1) * 128], esl,
                        start=first, stop=last,
                    )
                    nc.tensor.matmul(
                        st["av"][:], Vsb[kt][:, vsl], esl,
                        start=first, stop=last,
                    )
                if dkt == NDK - 1:
                    rec = rc.tile([128, QTS], F32, tag="rc", name="rc")
                    nc.vector.reciprocal_approx_fast(rec[:], st["dn"][:])
                    nc.vector.tensor_mul(
                        OT[h][:, (q % 2) * QTS : (q % 2 + 1) * QTS],
                        st["av"][:], rec[:]
                    )
                    del state[si]

            tasks = []
            for si in range(NQ * HL):
                for dkt in range(NDK):
                    scores_and_exp(si, dkt)
                    tasks.append((si, dkt))
                    if len(tasks) > LAG:
                        dn_av(*tasks.pop(0))
                if si >= HL:
                    og_emit(si - HL)
            while tasks:
                dn_av(*tasks.pop(0))
            for si in range(NQ * HL - HL, NQ * HL):
                og_emit(si)

    nc.compile()
    return nc


def _prep_in_maps(inputs):
    f32 = np.float32

    def bf(a):
        return np.ascontiguousarray(a).astype(BF16)

    x_r, x_i = np.asarray(inputs["x_r"], f32), np.asarray(inputs["x_i"], f32)
    ctx_r, ctx_i = np.asarray(inputs["ctx_r"], f32), np.asarray(inputs["ctx_i"], f32)
    mask = np.asarray(inputs["mask"], f32)
    W = {k: np.asarray(inputs[k], f32) for k in
         ("Wqr", "Wqi", "Wkr", "Wki", "Wvr", "Wvi", "Wor", "Woi")}

    per_batch = {}
    for b in range(B):
        def xtile(a):
            # [S, F] -> [F, S] -> [NFIN, NQ, 128, 512]
            return a.T.reshape(NFIN, 128, NQ, QTS).transpose(0, 2, 1, 3)

        tr, ti = xtile(x_r[b]), xtile(x_i[b])
        # [NFIN, NQ, 128, 2*QTS] with (r|i) per q, then fold q-pairs into rows
        xri = np.concatenate([tr, ti], axis=-1)
        xri = (
            xri.reshape(NFIN, NQ // 2, 2, 128, 2 * QTS)
            .transpose(0, 1, 3, 2, 4)
            .reshape(NFIN, NQ // 2, 128, 4 * QTS)
        )

        mcol = mask[b].reshape(NKT, 128).T  # [128, NKT]
        per_batch[b] = {
            "xT": bf(xri),
            "cTr": bf(ctx_r[b].T),
            "cTi": bf(ctx_i[b].T),
            "maskc": np.ascontiguousarray(mcol.astype(f32)),
            "maskb": bf(np.repeat(mcol, 128, axis=1)),
        }

    def merge_cols(Wr, Wi, g):
        """[Din, F] pair -> per-head merged column blocks.

        Returns (w1, w2) of shape [Din, HL*128]: per head h the 128 columns
        are [comp1_h(64) | comp2_h(64)] with w1 = [Wr_h | Wi_h] and
        w2 = [-Wi_h | Wr_h], so psum = w1^T xr + w2^T xi yields rows
        [real_h; imag_h]."""
        din = Wr.shape[0]
        w1 = np.empty((din, HL * 128), f32)
        w2 = np.empty((din, HL * 128), f32)
        for h in range(HL):
            cs = slice(g * FS + h * HD, g * FS + (h + 1) * HD)
            w1[:, h * 128 : h * 128 + 64] = Wr[:, cs]
            w1[:, h * 128 + 64 : (h + 1) * 128] = Wi[:, cs]
            w2[:, h * 128 : h * 128 + 64] = -Wi[:, cs]
            w2[:, h * 128 + 64 : (h + 1) * 128] = Wr[:, cs]
        return w1, w2

    def pack(w, nch, wid):
        # [nch*128, wid] -> packed [128, nch*wid]
        return bf(w.reshape(nch, 128, wid).transpose(1, 0, 2).reshape(128, -1))

    in_maps = []
    for core in range(NCORES):
        b, g = core // TPG, core % TPG
        m = dict(per_batch[b])
        # Gauss Q weights: plain per-core column slices of Wqr/Wqi/(Wqr+Wqi)
        gs = slice(g * FS, (g + 1) * FS)
        m["wqr"] = pack(W["Wqr"][:, gs], NFIN, FS)
        m["wqi"] = pack(W["Wqi"][:, gs], NFIN, FS)
        m["wqs"] = pack(W["Wqr"][:, gs] + W["Wqi"][:, gs], NFIN, FS)
        for pre, wr, wi, nch in (
            ("wk", "Wkr", "Wki", NDC),
            ("wv", "Wvr", "Wvi", NDC),
        ):
            w1, w2 = merge_cols(W[wr], W[wi], g)
            m[pre + "1"] = pack(w1, nch, WW)
            m[pre + "2"] = pack(w2, nch, WW)
        # Wo: rows re-ordered to the merged [out_r_h(64); out_i_h(64)] layout.
        wo1 = np.empty((HL, 128, F), f32)
        wo2 = np.empty((HL, 128, F), f32)
        for h in range(HL):
            rs = slice(g * FS + h * HD, g * FS + (h + 1) * HD)
            wo1[h, :64] = W["Wor"][rs]
            wo1[h, 64:] = -W["Woi"][rs]
            wo2[h, :64] = W["Woi"][rs]
            wo2[h, 64:] = W["Wor"][rs]
        m["wo1"] = bf(wo1.transpose(1, 0, 2).reshape(128, -1))
        m["wo2"] = bf(wo2.transpose(1, 0, 2).reshape(128, -1))
        in_maps.append(m)
    return in_maps


def kernel(**inputs):
    if "nc" not in _CACHE:
        _CACHE["nc"] = _build_nc()
    nc = _CACHE["nc"]
    in_maps = _prep_in_maps(inputs)
    res = run_bass_kernel_spmd(nc, in_maps, core_ids=list(range(NCORES)))
    y = np.zeros((B, S, F), np.complex64)
    for core in range(NCORES):
        b = core // TPG
        y[b] += res.results[core]["yr"].astype(np.float32)
        y[b] += 1j * res.results[core]["yi"].astype(np.float32)
    return y
